# revision 13
# baseline (speedup 1.0000x reference)
"""BEV pooling (Lift-Splat-Shoot scatter) Trainium2 kernel.

Strategy (8 NeuronCores = 4 batches x 2 y-halves):
  Geometry structure (identity rots/post_rots in this problem): the BEV cell
  of a frustum point depends only on (d, w); the z-keep mask only on (d, h).
  So per batch: h-reduce x[d,:,w,:] over kept h rows -> S1[(d,w), 80], then
  scatter-add ~9.4K columns into the 360x360x80 grid.

  Host (per kernel() call — the NEFF is compiled per invocation, so the whole
  schedule is static):
    - geometry via jax-on-CPU (bit-identical to the reference); masks, cells
    - per shard (batch, y-half): y-major linear cell ids (half 1 y-mirrored
      so both halves share one static schedule; host un-mirrors the output)
    - static window segmentation (DP over 512-cell atoms, window <= 2048
      cells) with per-window tile budgets = max over the 8 shards
    - x_pack gather: ragged flat [XLEN, 32*80] bf16 (per-tile lane count
      trimmed to the max over shards), zmask-dropped h rows zeroed, padded
      lanes zero with cell idx -1
  Device (per core, fully static instruction stream):
    - per tile: DMA x-tile [nl, 2560] bf16; h-reduce -> S1 [nl, 80] bf16
      (DVE tensor_reduce with fp32 internal accum, or GpSimd bf16 add-tree;
      statically load balanced); one-hot = is_equal(iota16, idx) -> bf16
      [nl, W] on DVE
    - per window: accumulate its tiles via one bf16 matmul per 512-chunk
      into PSUM [80, W] f32; ScalarE copy -> SBUF strip; DMA strip -> output
    - empty regions: DMA from a static zero strip
  Host: assemble output; mirror half-1 rows back.
"""

import numpy as np
import ml_dtypes

# ---------------- problem constants (hardcoded, self-contained) -------------
B, N = 4, 1
IH, IW = 256, 704
FH, FW = 32, 88
C = 80
XB = (-54.0, 54.0, 0.3)
YB = (-54.0, 54.0, 0.3)
ZB = (-10.0, 10.0, 20.0)
DB = (1.0, 60.0, 0.5)
D = int((DB[1] - DB[0]) / DB[2])          # 118
NXG = (360, 360, 1)
HALF = 180 * 360                           # cells per y-half
ATOM = 512
NATOMS = (HALF + ATOM - 1) // ATOM         # 127 (last atom short: 288)
MAXW_ATOMS = 4                             # window <= 2048 cells
SPAN = MAXW_ATOMS * ATOM
HC = FH * C                                # 2560

# x-DMA queue split: tile ti's DMA goes to the gpsimd (SWDGE) queue when
# ti % DMAQ_MOD == DMAQ_GPS (else sync queue)
DMAQ_MOD, DMAQ_GPS = 3, 2


def _geometry(inputs):
    """Frustum -> lidar-frame points, replicated from the reference.
    jax-on-CPU when available (bit-identical to the reference); numpy
    fallback (verified cell-identical on CPU)."""
    args = [np.asarray(inputs[k]) for k in
            ('rots', 'trans', 'intrins', 'post_rots', 'post_trans',
             'lidar2ego_rots', 'lidar2ego_trans', 'extra_rots', 'extra_trans')]
    try:
        import jax
        import jax.numpy as jnp
        cpu = jax.devices("cpu")[0]
        with jax.default_device(cpu):
            ds_ = jnp.broadcast_to(jnp.arange(DB[0], DB[1], DB[2], dtype=jnp.float32)[:, None, None], (D, FH, FW))
            xs = jnp.broadcast_to(jnp.linspace(0.0, IW - 1.0, FW, dtype=jnp.float32)[None, None, :], (D, FH, FW))
            ys = jnp.broadcast_to(jnp.linspace(0.0, IH - 1.0, FH, dtype=jnp.float32)[None, :, None], (D, FH, FW))
            frustum = jnp.stack([xs, ys, ds_], axis=-1)
            rots, trans, intrins, post_rots, post_trans, l2c_rots, l2c_trans, extra_rots, extra_trans = map(jnp.asarray, args)
            pts = frustum[None, None] - post_trans[:, :, None, None, None, :]
            pts = jnp.einsum('bnij,bndhwj->bndhwi', jnp.linalg.inv(post_rots), pts)
            pts = jnp.concatenate([pts[..., :2] * pts[..., 2:3], pts[..., 2:3]], axis=-1)
            combine = jnp.einsum('bnij,bnjk->bnik', rots, jnp.linalg.inv(intrins))
            pts = jnp.einsum('bnij,bndhwj->bndhwi', combine, pts) + trans[:, :, None, None, None, :]
            pts = pts - l2c_trans[:, None, None, None, None, :]
            pts = jnp.einsum('bij,bndhwj->bndhwi', jnp.linalg.inv(l2c_rots), pts)
            pts = jnp.einsum('bij,bndhwj->bndhwi', extra_rots, pts) + extra_trans[:, None, None, None, None, :]
            return np.asarray(pts)
    except Exception:
        pass
    rots, trans, intrins, post_rots, post_trans, l2c_rots, l2c_trans, extra_rots, extra_trans = \
        [a.astype(np.float32) for a in args]
    ds_ = np.broadcast_to(np.arange(DB[0], DB[1], DB[2], dtype=np.float32)[:, None, None], (D, FH, FW))
    xs = np.broadcast_to(np.linspace(0.0, IW - 1.0, FW, dtype=np.float32)[None, None, :], (D, FH, FW))
    ys = np.broadcast_to(np.linspace(0.0, IH - 1.0, FH, dtype=np.float32)[None, :, None], (D, FH, FW))
    frustum = np.stack([xs, ys, ds_], axis=-1)
    pts = frustum[None, None] - post_trans[:, :, None, None, None, :]
    pts = np.einsum('bnij,bndhwj->bndhwi', np.linalg.inv(post_rots), pts)
    pts = np.concatenate([pts[..., :2] * pts[..., 2:3], pts[..., 2:3]], axis=-1)
    combine = np.einsum('bnij,bnjk->bnik', rots, np.linalg.inv(intrins))
    pts = np.einsum('bnij,bndhwj->bndhwi', combine, pts) + trans[:, :, None, None, None, :]
    pts = pts - l2c_trans[:, None, None, None, None, :]
    pts = np.einsum('bij,bndhwj->bndhwi', np.linalg.inv(l2c_rots), pts)
    pts = np.einsum('bij,bndhwj->bndhwi', extra_rots, pts) + extra_trans[:, None, None, None, None, :]
    return pts.astype(np.float32)


def _plan_and_pack(inputs):
    x = np.asarray(inputs['x'])
    geom = _geometry(inputs)                                   # [B,1,D,FH,FW,3]
    DXv = np.array([XB[2], YB[2], ZB[2]], np.float32)
    BXv = np.array([XB[0] + XB[2] / 2, YB[0] + YB[2] / 2, ZB[0] + ZB[2] / 2], np.float32)
    coords = ((geom - (BXv - DXv / 2.0)) / DXv).astype(np.int32)

    cxy = coords[:, 0, :, 0, :, :2]                            # [B, D, FW, 2] (h-indep)
    cz = coords[:, 0, :, :, 0, 2]                              # [B, D, FH]   (w-indep)
    assert (coords[..., 0] == coords[:, :, :, :1, :, 0]).all()
    assert (coords[..., 1] == coords[:, :, :, :1, :, 1]).all()
    assert (coords[..., 2] == coords[:, :, :, :, :1, 2]).all()

    xym = ((cxy[..., 0] >= 0) & (cxy[..., 0] < NXG[0]) &
           (cxy[..., 1] >= 0) & (cxy[..., 1] < NXG[1]))        # [B, D, FW]
    zm = (cz == 0)                                             # [B, D, FH]

    shard_cols = []
    for b in range(B):
        dk, wk = np.nonzero(xym[b])
        cx = cxy[b, dk, wk, 0].astype(np.int64)
        cy = cxy[b, dk, wk, 1].astype(np.int64)
        for half in range(2):
            sel = (cy >= 180 * half) & (cy < 180 * (half + 1))
            cy2 = cy[sel] - 180 * half if half == 0 else 359 - cy[sel]
            lin = cy2 * 360 + cx[sel]
            order = np.argsort(lin, kind='stable')
            shard_cols.append((lin[order], dk[sel][order], wk[sel][order]))

    atom_counts = np.zeros((8, NATOMS), np.int64)
    for s, (lin, _, _) in enumerate(shard_cols):
        w_, c_ = np.unique(lin // ATOM, return_counts=True)
        atom_counts[s, w_] = c_
    pref = np.concatenate([np.zeros((8, 1), np.int64),
                           np.cumsum(atom_counts, axis=1)], axis=1)

    # DP segmentation over the whole half (no slot constraint)
    INF = 1 << 40
    dp = np.full(NATOMS + 1, INF, np.int64)
    dp[0] = 0
    ch = np.zeros(NATOMS + 1, np.int64)
    for i in range(1, NATOMS + 1):
        for w_ in range(1, min(MAXW_ATOMS, i) + 1):
            cols = pref[:, i] - pref[:, i - w_]
            cost = 0 if cols.max() == 0 else int(np.ceil(cols / 128).max())
            if dp[i - w_] + cost < dp[i]:
                dp[i] = dp[i - w_] + cost
                ch[i] = w_
    segs = []
    i = NATOMS
    while i > 0:
        w_ = ch[i]
        segs.append((i - w_, i))
        i -= w_
    segs = segs[::-1]

    # windows: (cell_lo, cell_hi, n_tiles); empties: list of (cell_lo, cell_hi)
    windows = []
    empties = []
    for (sa, sb) in segs:
        clo, chi = sa * ATOM, min(sb * ATOM, HALF)
        cols = pref[:, sb] - pref[:, sa]
        t = 0 if cols.max() == 0 else int(np.ceil(cols / 128).max())
        if t > 0:
            windows.append((clo, chi, t))
        else:
            empties.append((clo, chi))
    # merge adjacent empties, then chunk to <= SPAN
    merged = []
    for (a, bb) in empties:
        if merged and merged[-1][1] == a:
            merged[-1][1] = bb
        else:
            merged.append([a, bb])
    empties = []
    for (a, bb) in merged:
        while a < bb:
            e = min(a + SPAN, bb)
            empties.append((a, e))
            a = e
    # interleave heavy/light windows: spreads the multi-tile windows' deep
    # PSUM accumulation chains between the cheap single-tile windows so no
    # engine sees a long run of same-shaped work.
    windows.sort(key=lambda w: -w[2])
    half1, half2 = windows[: len(windows) // 2], windows[len(windows) // 2:]
    inter = []
    for i in range(max(len(half1), len(half2))):
        if i < len(half1):
            inter.append(half1[i])
        if i < len(half2):
            inter.append(half2[i])
    windows = inter
    NT = sum(t for _, _, t in windows)

    # per-tile max lane count across shards (static DMA/compute partition count)
    nlmax = np.zeros(NT, np.int64)
    for s in range(8):
        lin = shard_cols[s][0]
        ti = 0
        for (clo, chi, t) in windows:
            m0 = np.searchsorted(lin, clo, side='left')
            m1 = np.searchsorted(lin, chi, side='left')
            for k in range(t):
                nl = min(m0 + (k + 1) * 128, m1) - (m0 + k * 128)
                nlmax[ti] = max(nlmax[ti], max(0, nl))
                ti += 1
    nlmax = np.maximum(nlmax, 1)
    offs = np.concatenate([[0], np.cumsum(nlmax)]).astype(np.int64)
    XLEN = int(offs[-1])

    # partition placement: choose each x-tile's SBUF partition offset p0 so
    # the 16 SDMA engines (engine g <-> partitions 8g..8g+7) see balanced
    # byte loads. Strip writes (output, partitions 0..79) pre-load engines
    # 0-9, so x tiles lean toward high partitions.
    eng_load = np.zeros(16, np.float64)
    for (clo, chi, t) in windows:
        W = chi - clo
        eng_load[:10] += 8 * W * 2.0          # bf16 strip out-DMA reads SBUF p0..79
    p0s = np.zeros(NT, np.int64)
    for ti in range(NT):
        nl = int(nlmax[ti])
        best, bestcost = 0, None
        for p0 in (0, 32, 64):
            if p0 + nl > 128:
                continue
            trial = eng_load.copy()
            for g in range(16):
                ov = max(0, min(8 * g + 8, p0 + nl) - max(8 * g, p0))
                trial[g] += ov * HC * 2.0
            cost = trial.max()
            if bestcost is None or cost < bestcost:
                bestcost, best = cost, p0
        p0s[ti] = best
        for g in range(16):
            ov = max(0, min(8 * g + 8, best + nl) - max(8 * g, best))
            eng_load[g] += ov * HC * 2.0

    x_pack = np.zeros((8, XLEN, HC), ml_dtypes.bfloat16)
    idxs = np.full((8, 128, NT), -1.0, np.float32)
    xf = x.reshape(B, D, FH, FW, C)
    for s in range(8):
        b = s // 2
        lin, dk, wk = shard_cols[s]
        zmb = zm[b]
        ti = 0
        for (clo, chi, t) in windows:
            m0 = np.searchsorted(lin, clo, side='left')
            m1 = np.searchsorted(lin, chi, side='left')
            for k in range(t):
                lo = m0 + k * 128
                hi = min(m0 + (k + 1) * 128, m1)
                nl = max(0, hi - lo)
                if nl > 0:
                    dsel = dk[lo:hi]
                    wsel = wk[lo:hi]
                    blk = xf[b, dsel, :, wsel, :]              # [nl, FH, C]
                    blk = blk * zmb[dsel][:, :, None]
                    # [h][c] layout: the on-device add-tree halves over h
                    x_pack[s, offs[ti]:offs[ti] + nl] = blk.reshape(nl, HC).astype(ml_dtypes.bfloat16)
                    p0 = int(p0s[ti])
                    idxs[s, p0:p0 + nl, ti] = (lin[lo:hi] - clo).astype(np.float32)
                ti += 1
        assert ti == NT
    iota16 = np.broadcast_to(np.arange(SPAN, dtype=np.int16)[None, :],
                             (128, SPAN)).copy()
    return windows, empties, NT, nlmax, offs, XLEN, p0s, x_pack, idxs, iota16


def _build_program(windows, empties, NT, nlmax, offs, XLEN, p0s):
    import concourse.mybir as mybir
    import concourse.tile as tile
    from concourse import bacc

    F32, BF16, I16 = mybir.dt.float32, mybir.dt.bfloat16, mybir.dt.int16

    nc = bacc.Bacc("TRN2", target_bir_lowering=False, debug=False)
    x_d = nc.dram_tensor("xp", [XLEN, HC], BF16, kind="ExternalInput").ap()
    idx_d = nc.dram_tensor("idx", [128, NT], F32, kind="ExternalInput").ap()
    iota_d = nc.dram_tensor("iota", [128, SPAN], I16, kind="ExternalInput").ap()
    out_d = nc.dram_tensor("out", [C, HALF], F32, kind="ExternalOutput").ap()

    with tile.TileContext(nc) as tc:
        with (
            tc.tile_pool(name="persist", bufs=1) as persist,
            tc.tile_pool(name="xt", bufs=14) as xpool,
            tc.tile_pool(name="oh", bufs=12) as ohpool,
            tc.tile_pool(name="s1", bufs=24) as s1pool,
            tc.tile_pool(name="strip", bufs=4) as stpool,
            tc.tile_pool(name="psum", bufs=2, space="PSUM") as pspool,
        ):
            iota_t = persist.tile([128, SPAN], I16)
            idx_t = persist.tile([128, NT], F32)
            nc.sync.dma_start(iota_t[:], iota_d)
            nc.sync.dma_start(idx_t[:], idx_d)
            # empty output regions need no writes: PJRT donates pre-zeroed
            # output buffers, so untouched cells are already 0.

            ti = 0
            for wi, (clo, chi, t) in enumerate(windows):
                W = chi - clo
                ps = pspool.tile([C, SPAN], F32, tag="ps")
                for k in range(t):
                    nl = int(nlmax[ti])
                    off = int(offs[ti])
                    p0 = int(p0s[ti])
                    psl = slice(p0, p0 + nl)
                    xt = xpool.tile([128, HC], BF16, tag="xt")
                    nc.sync.dma_start(xt[psl], x_d[off:off + nl])
                    # one-hot first: no dependency on the x tile, so PE can
                    # start right after the folds land
                    oh = ohpool.tile([128, SPAN], BF16, tag="oh")
                    nc.vector.tensor_scalar(
                        out=oh[psl, :W], in0=iota_t[psl, :W],
                        scalar1=idx_t[psl, ti:ti + 1], scalar2=None,
                        op0=mybir.AluOpType.is_equal)
                    # bf16 add-tree over h ([h][c] layout): folds 1-3 on DVE
                    # (2x perf mode), tail folds 4-5 on the otherwise-idle
                    # GpSimd; the last fold writes a small dedicated S1 tile
                    # so the big xt DMA buffer frees at GPS fold5
                    s1 = s1pool.tile([128, C], BF16, tag="s1")
                    for w in (HC, HC // 2, HC // 4):
                        h_ = w // 2
                        nc.vector.tensor_tensor(
                            out=xt[psl, :h_], in0=xt[psl, :h_], in1=xt[psl, h_:w],
                            op=mybir.AluOpType.add)
                    nc.gpsimd.tensor_tensor(
                        out=xt[psl, :2 * C], in0=xt[psl, :2 * C], in1=xt[psl, 2 * C:4 * C],
                        op=mybir.AluOpType.add)
                    nc.gpsimd.tensor_tensor(
                        out=s1[psl], in0=xt[psl, :C], in1=xt[psl, C:2 * C],
                        op=mybir.AluOpType.add)
                    s1t = s1[psl]
                    nchunk = (W + 511) // 512
                    for cch in range(nchunk):
                        sl = slice(cch * 512, min((cch + 1) * 512, W))
                        nc.tensor.matmul(out=ps[:, sl], lhsT=s1t, rhs=oh[psl, sl],
                                         start=(k == 0), stop=(k == t - 1))
                    ti += 1
                strip = stpool.tile([C, SPAN], BF16, tag="strip")
                nc.scalar.activation(out=strip[:, :W], in_=ps[:, :W],
                                     func=mybir.ActivationFunctionType.Copy)
                # gpsimd (SWDGE) DMA casts bf16 -> f32 on the way to DRAM;
                # halves the SBUF-side DMA-engine load of the output writes
                nc.gpsimd.dma_start(out_d[:, clo:chi], strip[:, :W])
            assert ti == NT
    nc.compile()
    return nc


LAST_RESULTS = None
LAST_NC = None


def kernel(**inputs) -> np.ndarray:
    from concourse.bass_utils import run_bass_kernel_spmd

    windows, empties, NT, nlmax, offs, XLEN, p0s, x_pack, idxs, iota16 = _plan_and_pack(inputs)
    nc = _build_program(windows, empties, NT, nlmax, offs, XLEN, p0s)
    in_maps = [{"xp": x_pack[s], "idx": idxs[s], "iota": iota16} for s in range(8)]
    res = run_bass_kernel_spmd(nc, in_maps, core_ids=list(range(8)))
    global LAST_RESULTS, LAST_NC
    LAST_RESULTS = res
    LAST_NC = nc
    out = np.empty((B, C, 360, 360), np.float32)
    for b in range(B):
        lo = res.results[2 * b]["out"].reshape(C, 180, 360)
        hi = res.results[2 * b + 1]["out"].reshape(C, 180, 360)
        out[b, :, :180] = lo
        out[b, :, 180:] = hi[:, ::-1, :]
    return out
